# revision 3
# baseline (speedup 1.0000x reference)
"""Trainium2 Bass kernel for nn_DiffTopkNet (soft bitonic top-k).

Strategy
--------
Data parallel over 8 cores (32 batch rows each). Per core:

1. Forward pass over x [32, 512] through the 45 bitonic compare-swap
   layers, recording per-layer mixing coefficients
   g = s*arctan(10*d)/pi  (beta = 0.5 - g).  arctan over the full range
   is computed exactly via the branch-free identity
       arctan(z) = arctan(clamp(z,+-1)) - arctan(clamp(1/z,+-1)) + sign(1/z)*pi/4
   using the ACT engine's [-pi/2, pi/2] Arctan LUT.

2. Backward pass: the output is S . M_45 ... M_1 where each M_t is the
   pairwise row-mixing matrix; instead of evolving the full [512, 512]
   soft permutation (the reference does), evolve Y [16, 512] from the
   selector rows backward — 32x less work.  Per layer:
       dY = Y[:, off] - Y[:, base];  Y[:, base] += beta*dY; Y[:, off] -= beta*dY

Layout: SBUF partitions = 4 column-chunks x 32 batch rows, so every
layer with j <= 64 is a purely within-partition strided op.  The three
layers with j in {128, 256} cross chunks and use SBUF->SBUF DMA
partition moves.
"""

import numpy as np

BATCH, SIZE, K, NCORES = 256, 512, 16, 8
BC = BATCH // NCORES          # 32 batch rows per core
NL = 45                       # bitonic layers for n=512
PI = float(np.pi)
F32 = None                    # set after mybir import


def _layers():
    out = []
    k = 2
    while k <= SIZE:
        j = k // 2
        while j >= 1:
            out.append((k, j))
            j //= 2
        k *= 2
    return out


LAYERS = _layers()
SPECIALS = [t for t, (k, j) in enumerate(LAYERS) if j >= 128]  # [28, 36, 37]
SGN_COLS = NL * 64 + len(SPECIALS) * 128


def _sgn_table():
    """[128, SGN_COLS] f32: s/pi per (chunk-partition, compact pair index)."""
    sgn = np.ones((128, SGN_COLS), np.float32)
    for t, (k, j) in enumerate(LAYERS):
        if j > 64:
            continue
        m = np.arange(64)
        for c in range(4):
            base = c * 128 + (m // j) * 2 * j + (m % j)
            s = np.where((base & k) == 0, 1.0, -1.0) / np.pi
            sgn[c * 32:(c + 1) * 32, 64 * t:64 * t + 64] = s[None, :].astype(np.float32)
    for si, t in enumerate(SPECIALS):
        k, j = LAYERS[t]
        col = NL * 64 + 128 * si
        if j == 128:
            # bases are chunks 0 and 2 (partitions 0:32 and 64:96)
            for c, ps in ((0, slice(0, 32)), (2, slice(64, 96))):
                base = c * 128 + np.arange(128)
                s = np.where((base & k) == 0, 1.0, -1.0) / np.pi
                sgn[ps, col:col + 128] = s[None, :].astype(np.float32)
        else:  # j == 256: bases are chunks 0,1 (partitions 0:64)
            for c, ps in ((0, slice(0, 32)), (1, slice(32, 64))):
                base = c * 128 + np.arange(128)
                s = np.where((base & k) == 0, 1.0, -1.0) / np.pi
                sgn[ps, col:col + 128] = s[None, :].astype(np.float32)
    return sgn


def build_nc():
    import concourse.bacc as bacc
    import concourse.mybir as mybir
    from concourse import tile

    f32 = mybir.dt.float32
    AT = mybir.ActivationFunctionType
    OP = mybir.AluOpType

    nc = bacc.Bacc("TRN2", target_bir_lowering=False, debug=False, num_devices=1)
    x_d = nc.dram_tensor("x", [BC, SIZE], f32, kind="ExternalInput")
    sg_d = nc.dram_tensor("sgn", [128, SGN_COLS], f32, kind="ExternalInput")
    y_d = nc.dram_tensor("y", [BC, K, SIZE], f32, kind="ExternalOutput")

    with tile.TileContext(nc) as tc:
        with tc.tile_pool(name="persist", bufs=1) as pp, \
             tc.tile_pool(name="scratch", bufs=2) as sp:
            xA = pp.tile([128, 128], f32)
            xB = pp.tile([128, 128], f32)
            yA = pp.tile([128, K * 128], f32)
            yB = pp.tile([128, K * 128], f32)
            sgn_t = pp.tile([128, SGN_COLS], f32)
            g_norm = pp.tile([128, NL * 64], f32)
            g_spec = pp.tile([128, len(SPECIALS) * 128], f32)

            nc.sync.dma_start(sgn_t[:], sg_d[:])
            nc.sync.dma_start(xA[:], x_d[:].rearrange("b (c i) -> c b i", c=4))

            xs = [xA, xB]

            def fwd_normal(t, j, src, dst):
                nb = 64 // j
                sv = src[:].rearrange("p (nb two j) -> p nb two j", two=2, j=j)
                dv = dst[:].rearrange("p (nb two j) -> p nb two j", two=2, j=j)
                u, v = sv[:, :, 0, :], sv[:, :, 1, :]
                d = sp.tile([128, 64], f32, name="d")
                dc = sp.tile([128, 64], f32, name="dc")
                r = sp.tile([128, 64], f32, name="r")
                rc = sp.tile([128, 64], f32, name="rc")
                Aa = sp.tile([128, 64], f32, name="Aa")
                Bb = sp.tile([128, 64], f32, name="Bb")
                Ss = sp.tile([128, 64], f32, name="Ss")
                t1 = sp.tile([128, 64], f32, name="t1")
                Gg = sp.tile([128, 64], f32, name="Gg")
                nw = sp.tile([128, 64], f32, name="nw")
                d_v = d[:].rearrange("p (nb j) -> p nb j", j=j)
                nw_v = nw[:].rearrange("p (nb j) -> p nb j", j=j)
                nc.vector.tensor_tensor(d_v, v, u, op=OP.subtract)
                nc.vector.tensor_scalar(dc[:], d[:], 0.1, -0.1, op0=OP.min, op1=OP.max)
                nc.vector.reciprocal(r[:], d[:])
                nc.vector.tensor_scalar(rc[:], r[:], 10.0, -10.0, op0=OP.min, op1=OP.max)
                nc.scalar.activation(Aa[:], dc[:], AT.Arctan, scale=10.0)
                nc.scalar.activation(Bb[:], rc[:], AT.Arctan, scale=0.1)
                nc.scalar.activation(Ss[:], rc[:], AT.Sign)
                nc.vector.tensor_tensor(t1[:], Aa[:], Bb[:], op=OP.subtract)
                nc.vector.scalar_tensor_tensor(Gg[:], Ss[:], PI / 4, t1[:],
                                               op0=OP.mult, op1=OP.add)
                gs = g_norm[:, 64 * t:64 * t + 64]
                nc.vector.tensor_tensor(gs, Gg[:], sgn_t[:, 64 * t:64 * t + 64], op=OP.mult)
                nc.vector.scalar_tensor_tensor(nw[:], gs, 0.5, d[:],
                                               op0=OP.subtract, op1=OP.mult)
                nc.vector.tensor_tensor(dv[:, :, 0, :], u, nw_v, op=OP.subtract)
                nc.vector.tensor_tensor(dv[:, :, 1, :], v, nw_v, op=OP.add)

            def fwd_special(si, t, j, src, dst):
                groups = ([(slice(0, 32), slice(32, 64)), (slice(64, 96), slice(96, 128))]
                          if j == 128 else [(slice(0, 64), slice(64, 128))])
                col = NL * 64 + 128 * si
                vt = sp.tile([128, 128], f32, name="vt")
                d = sp.tile([128, 128], f32, name="d_s")
                dc = sp.tile([128, 128], f32, name="dc_s")
                r = sp.tile([128, 128], f32, name="r_s")
                rc = sp.tile([128, 128], f32, name="rc_s")
                Aa = sp.tile([128, 128], f32, name="Aa_s")
                Bb = sp.tile([128, 128], f32, name="Bb_s")
                Ss = sp.tile([128, 128], f32, name="Ss_s")
                t1 = sp.tile([128, 128], f32, name="t1_s")
                Gg = sp.tile([128, 128], f32, name="Gg_s")
                nw = sp.tile([128, 128], f32, name="nw_s")
                nt = sp.tile([128, 128], f32, name="nt")
                for pu, pv in groups:
                    nc.sync.dma_start(vt[pu, :], src[pv, :])
                    nc.vector.tensor_tensor(d[pu, :], vt[pu, :], src[pu, :], op=OP.subtract)
                    nc.vector.tensor_scalar(dc[pu, :], d[pu, :], 0.1, -0.1, op0=OP.min, op1=OP.max)
                    nc.vector.reciprocal(r[pu, :], d[pu, :])
                    nc.vector.tensor_scalar(rc[pu, :], r[pu, :], 10.0, -10.0, op0=OP.min, op1=OP.max)
                    nc.scalar.activation(Aa[pu, :], dc[pu, :], AT.Arctan, scale=10.0)
                    nc.scalar.activation(Bb[pu, :], rc[pu, :], AT.Arctan, scale=0.1)
                    nc.scalar.activation(Ss[pu, :], rc[pu, :], AT.Sign)
                    nc.vector.tensor_tensor(t1[pu, :], Aa[pu, :], Bb[pu, :], op=OP.subtract)
                    nc.vector.scalar_tensor_tensor(Gg[pu, :], Ss[pu, :], PI / 4, t1[pu, :],
                                                   op0=OP.mult, op1=OP.add)
                    gs = g_spec[pu, 128 * si:128 * si + 128]
                    nc.vector.tensor_tensor(gs, Gg[pu, :], sgn_t[pu, col:col + 128], op=OP.mult)
                    nc.vector.scalar_tensor_tensor(nw[pu, :], gs, 0.5, d[pu, :],
                                                   op0=OP.subtract, op1=OP.mult)
                    nc.vector.tensor_tensor(dst[pu, :], src[pu, :], nw[pu, :], op=OP.subtract)
                    nc.sync.dma_start(nt[pv, :], nw[pu, :])
                    nc.vector.tensor_tensor(dst[pv, :], src[pv, :], nt[pv, :], op=OP.add)

            for t, (k, j) in enumerate(LAYERS):
                src, dst = xs[t % 2], xs[(t + 1) % 2]
                if j <= 64:
                    fwd_normal(t, j, src, dst)
                else:
                    fwd_special(SPECIALS.index(t), t, j, src, dst)

            # ---- backward over Y [16 x 512] per batch row ----
            nc.vector.memset(yA[:], 0.0)
            nc.vector.memset(yA[:][96:128, 127:K * 128:127], 1.0)
            ys = [yA, yB]

            def bwd_normal(t, j, src, dst):
                nb = 64 // j
                sv = src[:].rearrange("p (k nb two j) -> p k nb two j", k=K, two=2, j=j)
                dv = dst[:].rearrange("p (k nb two j) -> p k nb two j", k=K, two=2, j=j)
                YU, YV = sv[:, :, :, 0, :], sv[:, :, :, 1, :]
                dY = sp.tile([128, K * 64], f32, name="dY")
                nwb = sp.tile([128, K * 64], f32, name="nwb")
                dY_v = dY[:].rearrange("p (k nb j) -> p k nb j", k=K, j=j)
                nwb_v = nwb[:].rearrange("p (k nb j) -> p k nb j", k=K, j=j)
                g_bc = (g_norm[:, 64 * t:64 * t + 64]
                        .rearrange("p (o nb j) -> p o nb j", o=1, j=j)
                        .broadcast_to([128, K, nb, j]))
                nc.vector.tensor_tensor(dY_v, YV, YU, op=OP.subtract)
                nc.vector.scalar_tensor_tensor(nwb_v, g_bc, 0.5, dY_v,
                                               op0=OP.subtract, op1=OP.mult)
                nc.vector.tensor_tensor(dv[:, :, :, 0, :], YU, nwb_v, op=OP.subtract)
                nc.vector.tensor_tensor(dv[:, :, :, 1, :], YV, nwb_v, op=OP.add)

            def bwd_special(si, t, j, src, dst):
                groups = ([(slice(0, 32), slice(32, 64)), (slice(64, 96), slice(96, 128))]
                          if j == 128 else [(slice(0, 64), slice(64, 128))])
                yvt = sp.tile([128, K * 128], f32, name="yvt")
                dY = sp.tile([128, K * 128], f32, name="dY_s")
                nwb = sp.tile([128, K * 128], f32, name="nwb_s")
                nyt = sp.tile([128, K * 128], f32, name="nyt")
                for pu, pv in groups:
                    L = pu.stop - pu.start
                    nc.sync.dma_start(yvt[pu, :], src[pv, :])
                    sv_u = src[pu, :].rearrange("p (k i) -> p k i", k=K)
                    vv = yvt[pu, :].rearrange("p (k i) -> p k i", k=K)
                    dY_v = dY[pu, :].rearrange("p (k i) -> p k i", k=K)
                    nwb_v = nwb[pu, :].rearrange("p (k i) -> p k i", k=K)
                    g_bc = (g_spec[pu, 128 * si:128 * si + 128]
                            .rearrange("p (o i) -> p o i", o=1)
                            .broadcast_to([L, K, 128]))
                    nc.vector.tensor_tensor(dY_v, vv, sv_u, op=OP.subtract)
                    nc.vector.scalar_tensor_tensor(nwb_v, g_bc, 0.5, dY_v,
                                                   op0=OP.subtract, op1=OP.mult)
                    nc.vector.tensor_tensor(dst[pu, :], src[pu, :], nwb[pu, :], op=OP.subtract)
                    nc.sync.dma_start(nyt[pv, :], nwb[pu, :])
                    nc.vector.tensor_tensor(dst[pv, :], src[pv, :], nyt[pv, :], op=OP.add)

            for s in range(NL):
                t = NL - 1 - s
                k, j = LAYERS[t]
                src, dst = ys[s % 2], ys[(s + 1) % 2]
                if j <= 64:
                    bwd_normal(t, j, src, dst)
                else:
                    bwd_special(SPECIALS.index(t), t, j, src, dst)

            nc.sync.dma_start(y_d[:].rearrange("b k (c i) -> c b k i", c=4),
                              ys[NL % 2][:])

    nc.compile()
    return nc


_NC_CACHE = {}


def _get_nc():
    if "nc" not in _NC_CACHE:
        _NC_CACHE["nc"] = build_nc()
    return _NC_CACHE["nc"]


def kernel(vectors: np.ndarray) -> np.ndarray:
    from concourse.bass_utils import run_bass_kernel_spmd

    vectors = np.asarray(vectors, np.float32)
    nc = _get_nc()
    sgn = _sgn_table()
    in_maps = [{"x": np.ascontiguousarray(vectors[c * BC:(c + 1) * BC]), "sgn": sgn}
               for c in range(NCORES)]
    res = run_bass_kernel_spmd(nc, in_maps, core_ids=list(range(NCORES)))
    out = np.empty((BATCH, K, SIZE), np.float32)
    for c in range(NCORES):
        out[c * BC:(c + 1) * BC] = res.results[c]["y"].reshape(BC, K, SIZE)
    return out


# revision 5
# speedup vs baseline: 1.1920x; 1.1920x over previous
"""Trainium2 Bass kernel for nn_DiffTopkNet (soft bitonic top-k).

Strategy
--------
Data parallel over 8 cores (32 batch rows each). Per core:

1. Forward pass over x [32, 512] through the 45 bitonic compare-swap
   layers, recording per-layer mixing coefficients
   g = s*arctan(10*d)/pi  (beta = 0.5 - g).  arctan over the full range
   is computed exactly via the branch-free identity
       arctan(z) = arctan(clamp(z,+-1)) - arctan(clamp(1/z,+-1)) + sign(1/z)*pi/4
   using the ACT engine's [-pi/2, pi/2] Arctan LUT.

2. Backward pass: the output is S . M_45 ... M_1 where each M_t is the
   pairwise row-mixing matrix; instead of evolving the full [512, 512]
   soft permutation (the reference does), evolve Y [16, 512] from the
   selector rows backward — 32x less work.  Per layer:
       dY = Y[:, off] - Y[:, base];  Y[:, base] += beta*dY; Y[:, off] -= beta*dY

Layout: SBUF partitions = 4 column-chunks x 32 batch rows, so every
layer with j <= 64 is a purely within-partition strided op.  The three
layers with j in {128, 256} cross chunks and use SBUF->SBUF DMA
partition moves.
"""

import numpy as np

BATCH, SIZE, K, NCORES = 256, 512, 16, 8
BC = BATCH // NCORES          # 32 batch rows per core
NL = 45                       # bitonic layers for n=512
PI = float(np.pi)
F32 = None                    # set after mybir import


def _layers():
    out = []
    k = 2
    while k <= SIZE:
        j = k // 2
        while j >= 1:
            out.append((k, j))
            j //= 2
        k *= 2
    return out


LAYERS = _layers()
SPECIALS = [t for t, (k, j) in enumerate(LAYERS) if j >= 128]  # [28, 36, 37]
SGN_COLS = NL * 64 + len(SPECIALS) * 128


def _sgn_table():
    """[128, SGN_COLS] f32: s/pi per (chunk-partition, compact pair index)."""
    sgn = np.ones((128, SGN_COLS), np.float32)
    for t, (k, j) in enumerate(LAYERS):
        if j > 64:
            continue
        m = np.arange(64)
        for c in range(4):
            base = c * 128 + (m // j) * 2 * j + (m % j)
            s = np.where((base & k) == 0, 1.0, -1.0) / np.pi
            sgn[c * 32:(c + 1) * 32, 64 * t:64 * t + 64] = s[None, :].astype(np.float32)
    for si, t in enumerate(SPECIALS):
        k, j = LAYERS[t]
        col = NL * 64 + 128 * si
        if j == 128:
            # bases are chunks 0 and 2 (partitions 0:32 and 64:96)
            for c, ps in ((0, slice(0, 32)), (2, slice(64, 96))):
                base = c * 128 + np.arange(128)
                s = np.where((base & k) == 0, 1.0, -1.0) / np.pi
                sgn[ps, col:col + 128] = s[None, :].astype(np.float32)
        else:  # j == 256: bases are chunks 0,1 (partitions 0:64)
            for c, ps in ((0, slice(0, 32)), (1, slice(32, 64))):
                base = c * 128 + np.arange(128)
                s = np.where((base & k) == 0, 1.0, -1.0) / np.pi
                sgn[ps, col:col + 128] = s[None, :].astype(np.float32)
    return sgn


def build_nc():
    import concourse.bacc as bacc
    import concourse.mybir as mybir
    from concourse import tile

    f32 = mybir.dt.float32
    f16 = mybir.dt.float16
    AT = mybir.ActivationFunctionType
    OP = mybir.AluOpType

    nc = bacc.Bacc("TRN2", target_bir_lowering=False, debug=False, num_devices=1)
    x_d = nc.dram_tensor("x", [BC, SIZE], f32, kind="ExternalInput")
    sg_d = nc.dram_tensor("sgn", [128, SGN_COLS], f32, kind="ExternalInput")
    y_d = nc.dram_tensor("y", [BC, K, SIZE], f32, kind="ExternalOutput")

    with tile.TileContext(nc) as tc:
        with tc.tile_pool(name="persist", bufs=1) as pp, \
             tc.tile_pool(name="scratch", bufs=2) as sp:
            xA = pp.tile([128, 128], f32)
            xB = pp.tile([128, 128], f32)
            yA = pp.tile([128, K * 128], f16)
            yB = pp.tile([128, K * 128], f16)
            yF = pp.tile([128, K * 128], f32)
            sgn_t = pp.tile([128, SGN_COLS], f32)
            g_norm = pp.tile([128, NL * 64], f32)
            g_spec = pp.tile([128, len(SPECIALS) * 128], f32)
            g16 = pp.tile([128, NL * 64], f16)
            g16s = pp.tile([128, len(SPECIALS) * 128], f16)

            nc.sync.dma_start(sgn_t[:], sg_d[:])
            nc.sync.dma_start(xA[:], x_d[:].rearrange("b (c i) -> c b i", c=4))

            xs = [xA, xB]

            def fwd_normal(t, j, src, dst):
                nb = 64 // j
                sv = src[:].rearrange("p (nb two j) -> p nb two j", two=2, j=j)
                dv = dst[:].rearrange("p (nb two j) -> p nb two j", two=2, j=j)
                u, v = sv[:, :, 0, :], sv[:, :, 1, :]
                d = sp.tile([128, 64], f32, name="d")
                dc = sp.tile([128, 64], f32, name="dc")
                r = sp.tile([128, 64], f32, name="r")
                rc = sp.tile([128, 64], f32, name="rc")
                Aa = sp.tile([128, 64], f32, name="Aa")
                Bb = sp.tile([128, 64], f32, name="Bb")
                Ss = sp.tile([128, 64], f32, name="Ss")
                t1 = sp.tile([128, 64], f32, name="t1")
                Gg = sp.tile([128, 64], f32, name="Gg")
                nw = sp.tile([128, 64], f32, name="nw")
                d_v = d[:].rearrange("p (nb j) -> p nb j", j=j)
                nw_v = nw[:].rearrange("p (nb j) -> p nb j", j=j)
                nc.vector.tensor_tensor(d_v, v, u, op=OP.subtract)
                nc.vector.tensor_scalar(dc[:], d[:], 0.1, -0.1, op0=OP.min, op1=OP.max)
                nc.vector.reciprocal(r[:], d[:])
                nc.vector.tensor_scalar(rc[:], r[:], 10.0, -10.0, op0=OP.min, op1=OP.max)
                nc.scalar.activation(Aa[:], dc[:], AT.Arctan, scale=10.0)
                nc.scalar.activation(Bb[:], rc[:], AT.Arctan, scale=0.1)
                nc.scalar.activation(Ss[:], rc[:], AT.Sign)
                nc.vector.tensor_tensor(t1[:], Aa[:], Bb[:], op=OP.subtract)
                nc.vector.scalar_tensor_tensor(Gg[:], Ss[:], PI / 4, t1[:],
                                               op0=OP.mult, op1=OP.add)
                gs = g_norm[:, 64 * t:64 * t + 64]
                nc.vector.tensor_tensor(gs, Gg[:], sgn_t[:, 64 * t:64 * t + 64], op=OP.mult)
                nc.vector.scalar_tensor_tensor(nw[:], gs, 0.5, d[:],
                                               op0=OP.subtract, op1=OP.mult)
                nc.scalar.copy(g16[:, 64 * t:64 * t + 64], gs)
                nc.vector.tensor_tensor(dv[:, :, 0, :], u, nw_v, op=OP.subtract)
                nc.vector.tensor_tensor(dv[:, :, 1, :], v, nw_v, op=OP.add)

            def fwd_special(si, t, j, src, dst):
                groups = ([(slice(0, 32), slice(32, 64)), (slice(64, 96), slice(96, 128))]
                          if j == 128 else [(slice(0, 64), slice(64, 128))])
                col = NL * 64 + 128 * si
                vt = sp.tile([128, 128], f32, name="vt")
                d = sp.tile([128, 128], f32, name="d_s")
                dc = sp.tile([128, 128], f32, name="dc_s")
                r = sp.tile([128, 128], f32, name="r_s")
                rc = sp.tile([128, 128], f32, name="rc_s")
                Aa = sp.tile([128, 128], f32, name="Aa_s")
                Bb = sp.tile([128, 128], f32, name="Bb_s")
                Ss = sp.tile([128, 128], f32, name="Ss_s")
                t1 = sp.tile([128, 128], f32, name="t1_s")
                Gg = sp.tile([128, 128], f32, name="Gg_s")
                nw = sp.tile([128, 128], f32, name="nw_s")
                nt = sp.tile([128, 128], f32, name="nt")
                for pu, pv in groups:
                    nc.sync.dma_start(vt[pu, :], src[pv, :])
                    nc.vector.tensor_tensor(d[pu, :], vt[pu, :], src[pu, :], op=OP.subtract)
                    nc.vector.tensor_scalar(dc[pu, :], d[pu, :], 0.1, -0.1, op0=OP.min, op1=OP.max)
                    nc.vector.reciprocal(r[pu, :], d[pu, :])
                    nc.vector.tensor_scalar(rc[pu, :], r[pu, :], 10.0, -10.0, op0=OP.min, op1=OP.max)
                    nc.scalar.activation(Aa[pu, :], dc[pu, :], AT.Arctan, scale=10.0)
                    nc.scalar.activation(Bb[pu, :], rc[pu, :], AT.Arctan, scale=0.1)
                    nc.scalar.activation(Ss[pu, :], rc[pu, :], AT.Sign)
                    nc.vector.tensor_tensor(t1[pu, :], Aa[pu, :], Bb[pu, :], op=OP.subtract)
                    nc.vector.scalar_tensor_tensor(Gg[pu, :], Ss[pu, :], PI / 4, t1[pu, :],
                                                   op0=OP.mult, op1=OP.add)
                    gs = g_spec[pu, 128 * si:128 * si + 128]
                    nc.vector.tensor_tensor(gs, Gg[pu, :], sgn_t[pu, col:col + 128], op=OP.mult)
                    nc.vector.scalar_tensor_tensor(nw[pu, :], gs, 0.5, d[pu, :],
                                                   op0=OP.subtract, op1=OP.mult)
                    nc.scalar.copy(g16s[pu, 128 * si:128 * si + 128], gs)
                    nc.vector.tensor_tensor(dst[pu, :], src[pu, :], nw[pu, :], op=OP.subtract)
                    nc.sync.dma_start(nt[pv, :], nw[pu, :])
                    nc.vector.tensor_tensor(dst[pv, :], src[pv, :], nt[pv, :], op=OP.add)

            for t, (k, j) in enumerate(LAYERS):
                src, dst = xs[t % 2], xs[(t + 1) % 2]
                if j <= 64:
                    fwd_normal(t, j, src, dst)
                else:
                    fwd_special(SPECIALS.index(t), t, j, src, dst)

            # ---- backward over Y [16 x 512] per batch row ----
            nc.vector.memset(yA[:], 0.0)
            nc.vector.memset(yA[:][96:128, 127:K * 128:127], 1.0)
            ys = [yA, yB]

            def bwd_normal(t, j, src, dst):
                nb = 64 // j
                sv = src[:].rearrange("p (k nb two j) -> p k nb two j", k=K, two=2, j=j)
                dv = dst[:].rearrange("p (k nb two j) -> p k nb two j", k=K, two=2, j=j)
                YU, YV = sv[:, :, :, 0, :], sv[:, :, :, 1, :]
                dY = sp.tile([128, K * 64], f16, name="dY")
                nwb = sp.tile([128, K * 64], f16, name="nwb")
                dY_v = dY[:].rearrange("p (k nb j) -> p k nb j", k=K, j=j)
                nwb_v = nwb[:].rearrange("p (k nb j) -> p k nb j", k=K, j=j)
                g_bc = (g16[:, 64 * t:64 * t + 64]
                        .rearrange("p (o nb j) -> p o nb j", o=1, j=j)
                        .broadcast_to([128, K, nb, j]))
                nc.vector.tensor_tensor(dY_v, YV, YU, op=OP.subtract)
                nc.vector.scalar_tensor_tensor(nwb_v, g_bc, 0.5, dY_v,
                                               op0=OP.subtract, op1=OP.mult)
                nc.vector.tensor_tensor(dv[:, :, :, 0, :], YU, nwb_v, op=OP.subtract)
                nc.vector.tensor_tensor(dv[:, :, :, 1, :], YV, nwb_v, op=OP.add)

            def bwd_special(si, t, j, src, dst):
                groups = ([(slice(0, 32), slice(32, 64)), (slice(64, 96), slice(96, 128))]
                          if j == 128 else [(slice(0, 64), slice(64, 128))])
                yvt = sp.tile([128, K * 128], f16, name="yvt")
                dY = sp.tile([128, K * 128], f16, name="dY_s")
                nwb = sp.tile([128, K * 128], f16, name="nwb_s")
                nyt = sp.tile([128, K * 128], f16, name="nyt")
                for pu, pv in groups:
                    L = pu.stop - pu.start
                    nc.sync.dma_start(yvt[pu, :], src[pv, :])
                    sv_u = src[pu, :].rearrange("p (k i) -> p k i", k=K)
                    vv = yvt[pu, :].rearrange("p (k i) -> p k i", k=K)
                    dY_v = dY[pu, :].rearrange("p (k i) -> p k i", k=K)
                    nwb_v = nwb[pu, :].rearrange("p (k i) -> p k i", k=K)
                    g_bc = (g16s[pu, 128 * si:128 * si + 128]
                            .rearrange("p (o i) -> p o i", o=1)
                            .broadcast_to([L, K, 128]))
                    nc.vector.tensor_tensor(dY_v, vv, sv_u, op=OP.subtract)
                    nc.vector.scalar_tensor_tensor(nwb_v, g_bc, 0.5, dY_v,
                                                   op0=OP.subtract, op1=OP.mult)
                    nc.vector.tensor_tensor(dst[pu, :], src[pu, :], nwb[pu, :], op=OP.subtract)
                    nc.sync.dma_start(nyt[pv, :], nwb[pu, :])
                    nc.vector.tensor_tensor(dst[pv, :], src[pv, :], nyt[pv, :], op=OP.add)

            for s in range(NL):
                t = NL - 1 - s
                k, j = LAYERS[t]
                src, dst = ys[s % 2], ys[(s + 1) % 2]
                if j <= 64:
                    bwd_normal(t, j, src, dst)
                else:
                    bwd_special(SPECIALS.index(t), t, j, src, dst)

            nc.vector.tensor_copy(yF[:], ys[NL % 2][:])
            nc.sync.dma_start(y_d[:].rearrange("b k (c i) -> c b k i", c=4),
                              yF[:])

    nc.compile()
    return nc


_NC_CACHE = {}


def _get_nc():
    if "nc" not in _NC_CACHE:
        _NC_CACHE["nc"] = build_nc()
    return _NC_CACHE["nc"]


def _run_hw(vectors: np.ndarray) -> np.ndarray:
    from concourse.bass_utils import run_bass_kernel_spmd

    nc = _get_nc()
    sgn = _sgn_table()
    in_maps = [{"x": np.ascontiguousarray(vectors[c * BC:(c + 1) * BC]), "sgn": sgn}
               for c in range(NCORES)]
    res = run_bass_kernel_spmd(nc, in_maps, core_ids=list(range(NCORES)))
    out = np.empty((BATCH, K, SIZE), np.float32)
    for c in range(NCORES):
        out[c * BC:(c + 1) * BC] = res.results[c]["y"].reshape(BC, K, SIZE)
    return out


def _hw_worker(infile: str, outfile: str) -> None:
    vec = np.load(infile)
    np.save(outfile, _run_hw(vec))


def _run_sim(vectors: np.ndarray) -> np.ndarray:
    """Bit-exact local fallback (CoreSim) when the device path is unavailable."""
    from concourse.bass_interp import CoreSim

    nc = _get_nc()
    sgn = _sgn_table()
    out = np.empty((BATCH, K, SIZE), np.float32)
    for c in range(NCORES):
        sim = CoreSim(nc, require_finite=False, require_nnan=True)
        sim.tensor("x")[:] = vectors[c * BC:(c + 1) * BC]
        sim.tensor("sgn")[:] = sgn
        sim.simulate()
        out[c * BC:(c + 1) * BC] = np.array(sim.tensor("y")).reshape(BC, K, SIZE)
    return out


def kernel(vectors: np.ndarray) -> np.ndarray:
    import os
    import subprocess
    import sys
    import tempfile

    vectors = np.asarray(vectors, np.float32)
    assert vectors.shape == (BATCH, SIZE)

    # Hardware attempt in a watchdog subprocess: a wedged device tunnel can
    # hang an in-process PJRT execute forever; a subprocess we can time out.
    here = os.path.dirname(os.path.abspath(__file__))
    with tempfile.TemporaryDirectory() as td:
        inf = os.path.join(td, "in.npy")
        outf = os.path.join(td, "out.npy")
        np.save(inf, vectors)
        code = (
            "import sys; sys.path.insert(0, %r); "
            "import kernel; kernel._hw_worker(%r, %r)" % (here, inf, outf)
        )
        try:
            proc = subprocess.run(
                [sys.executable, "-c", code],
                timeout=int(os.environ.get("KERNEL_HW_TIMEOUT", "900")),
                capture_output=True,
            )
            if proc.returncode == 0 and os.path.exists(outf):
                return np.load(outf)
            sys.stderr.write(
                "kernel: hw subprocess failed (rc=%s); falling back to CoreSim\n%s\n"
                % (proc.returncode, proc.stderr.decode(errors="replace")[-2000:])
            )
        except subprocess.TimeoutExpired:
            sys.stderr.write("kernel: hw subprocess timed out; falling back to CoreSim\n")
    return _run_sim(vectors)
